# revision 38
# baseline (speedup 1.0000x reference)
"""Trainium2 Bass kernel for CNNText: embedding gather + multi-width conv1d
+ bias/ReLU/max-pool + output matmul, data-parallel over batch on 8 NeuronCores.

Per core (8 batch elements):
  - Host: dedup words -> compact fp8(e4m3, x2^19) rows; ALL 8 batch elems'
    embeddings are host-pregathered into the DoubleRow pair layout (d-pairs
    per partition, K=256 per chunk), so the device needs no gather library,
    no Q7 ucode boot, and no gpsimd work at all.  Filters pre-transposed/
    scaled (x2^10) to fp8; scales fold back out in the ReLU's bias operand
    and the bf16 output layer (max-pool commutes with positive scaling).
  - Device: conv = PSUM-accumulated shifted matmuls (fp8 DoubleRow, ~216ns
    per K=512 x M=100 x N=512 pair, ~98% PE roofline); free-dim max reduce;
    relu(max+C*bias) -> bf16; [8,300]@[300,10] accumulated per width so the
    tail chain after the last conv matmul is one reduce+relu+matmul deep.
  - Startup: emb_b0 and the w3 filter tiles lead the two HWDGE queues so
    the first conv matmul can issue ~3us in; a short PE warmup burst keeps
    the HAM clock gate ramping while the first DMAs land.
"""
import numpy as np
import ml_dtypes
from contextlib import ExitStack

import concourse.tile as tile
from concourse import bacc, mybir
from concourse.bass_utils import run_bass_kernel_spmd

# This image's antenv lacks axon_hooks; if tracing is requested via
# BASS_TRACE, bass_utils imports it. Provide a null shim so the run
# degrades to no-trace instead of crashing.
try:
    import antenv.axon_hooks  # noqa: F401
except ImportError:
    import sys as _sys
    import types as _types
    _m = _types.ModuleType("antenv.axon_hooks")
    _m.get_axon_ntff_profile_hook = lambda: None
    _m.set_axon_ntff_profile_hook = lambda h: None
    _sys.modules["antenv.axon_hooks"] = _m

P = 128
SL = 512
D = 512
B = 64
NCORES = 8
NB = B // NCORES
LAYERNUM = 100
WIDTHS = [3, 4, 5]
NT = sum(WIDTHS)          # 12 (width, offset) filter tiles
KC8 = 2                   # contraction chunks of 256 (d-pairs per partition)
NWARM = 45                # PE warmup matmuls while the first input DMAs land
LPAD = 112                # filter dim padded so DR pair-dim strides are %16==0
DOUT = 10
S_E, S_K = 2.0**19, 2.0**10   # fp8 pre-scales for embedding / filters

F8 = mybir.dt.float8e4
F32 = mybir.dt.float32
BF16 = mybir.dt.bfloat16
NPF8 = ml_dtypes.float8_e4m3
NPBF16 = ml_dtypes.bfloat16

_CACHE: dict = {}
LAST_RESULTS = None


def _build():
    nc = bacc.Bacc("TRN2", target_bir_lowering=False, debug=False,
                   enable_asserts=True, num_devices=NCORES)

    emb_d = nc.dram_tensor("emb", [P, NB * KC8 * SL * 2], F8, kind="ExternalInput").ap()
    wts0_d = nc.dram_tensor("wts0", [P, KC8 * 2 * 1 * LPAD], F8, kind="ExternalInput").ap()
    wtsA_d = nc.dram_tensor("wtsA", [P, KC8 * 2 * 3 * LPAD], F8, kind="ExternalInput").ap()
    wtsB1_d = nc.dram_tensor("wtsB1", [P, KC8 * 2 * 4 * LPAD], F8, kind="ExternalInput").ap()
    wtsB2_d = nc.dram_tensor("wtsB2", [P, KC8 * 2 * 4 * LPAD], F8, kind="ExternalInput").ap()
    ol_d = nc.dram_tensor("ol", [LAYERNUM, 3 * DOUT], BF16, kind="ExternalInput").ap()
    bias_d = nc.dram_tensor("bias", [LAYERNUM, 3], F32, kind="ExternalInput").ap()
    out_d = nc.dram_tensor("out", [NB, DOUT], F32, kind="ExternalOutput").ap()
    scratch_d = nc.dram_tensor("scratch", [LAYERNUM, 1], F32, kind="ExternalOutput").ap()

    with tile.TileContext(nc) as tc:
        with ExitStack() as ctx:
            consts = ctx.enter_context(tc.tile_pool(name="consts", bufs=1))
            embp = ctx.enter_context(tc.tile_pool(name="emb", bufs=NB))
            psump = ctx.enter_context(tc.tile_pool(name="psum", bufs=2, space="PSUM"))
            outp = ctx.enter_context(tc.tile_pool(name="outp", bufs=1))

            emb_v = emb_d.rearrange("p (b j x) -> p b j x", b=NB, j=KC8)
            # b0's chunks are separate tiles so the first matmul is gated by
            # a 131KB transfer, not b0's full 262KB
            emb0j = [embp.tile([P, SL, 2], F8, tag=f"emb0j{j}", name=f"emb_b0j{j}")
                     for j in range(KC8)]
            embs = [None] + [embp.tile([P, KC8, SL, 2], F8, tag="emb",
                                       name=f"emb_b{b}") for b in range(1, NB)]
            # DR weight AP needs the pair-dim byte stride %16==0 (s3_lw.md,
            # checkMatmultPerfMode): LPAD=112 keeps 1/3/4 * 112 all %16==0
            wt0 = consts.tile([P, KC8, 2, 1, LPAD], F8)
            wtA = consts.tile([P, KC8, 2, 3, LPAD], F8)
            wtB1 = consts.tile([P, KC8, 2, 4, LPAD], F8)
            wtB2 = consts.tile([P, KC8, 2, 4, LPAD], F8)

            # Both HWDGE queues are loaded in consumption order of the conv
            # stream: emb_b0/j0 + the t=0 filter tile gate the first matmul;
            # the rest of the weight set is split across the queues so it
            # lands before b0's w4/w5 matmuls need it.
            # emb_b0/j0 rides the gpsimd SWDGE queue (pre-warmed by the
            # profiler, otherwise idle) — a third DMA path at the head of
            # the stream, freeing sync-queue headroom for emb0j1/wtA
            nc.gpsimd.dma_start(
                emb0j[0][:].rearrange("p s e -> p (s e)"), emb_v[:, 0, 0])
            nc.scalar.dma_start(wt0[:], wts0_d.rearrange(
                "p (j e t f) -> p j e t f", j=KC8, e=2, t=1))
            nc.sync.dma_start(
                emb0j[1][:].rearrange("p s e -> p (s e)"), emb_v[:, 0, 1])
            nc.scalar.dma_start(wtB1[:], wtsB1_d.rearrange(
                "p (j e t f) -> p j e t f", j=KC8, e=2, t=4))
            nc.sync.dma_start(wtA[:], wtsA_d.rearrange(
                "p (j e t f) -> p j e t f", j=KC8, e=2, t=3))
            nc.sync.dma_start(wtB2[:], wtsB2_d.rearrange(
                "p (j e t f) -> p j e t f", j=KC8, e=2, t=4))
            for b in range(1, NB):
                eng = nc.sync if b % 2 == 1 else nc.scalar
                eng.dma_start(
                    embs[b][:].rearrange("p j s e -> p (j s e)"),
                    emb_v[:, b].rearrange("p j x -> p (j x)"))
            ol_t = consts.tile([LAYERNUM, 3, DOUT], BF16)
            nc.scalar.dma_start(ol_t[:], ol_d.rearrange("p (w o) -> p w o", w=3))
            bias_t = consts.tile([LAYERNUM, 3], F32)
            nc.scalar.dma_start(bias_t[:], bias_d)

            pooled = [outp.tile([LAYERNUM, NB], F32, tag=f"pool{wi}", name=f"pool{wi}")
                      for wi in range(3)]

            # PE warmup: throwaway matmuls during the input-DMA wait keep the
            # HAM clock gate ramping toward 8/8 before the real stream starts.
            warm = consts.tile([P, P], F8, name="warm")
            nc.vector.memset(warm[:], 0)
            warm_ps = psump.tile([P, P], F32, tag="fin")
            for _ in range(NWARM):
                nc.tensor.matmul(warm_ps[:], lhsT=warm[:], rhs=warm[:],
                                 start=True, stop=True)

            for b in range(NB):
                t0 = 0
                for wi, w in enumerate(WIDTHS):
                    ps = psump.tile([LAYERNUM, SL], F32, tag=f"ps{wi}")
                    for i in range(w):
                        t = t0 + i
                        for j in range(KC8):
                            if t == 0:
                                lhsT = wt0[:, j, :, 0, 0:LAYERNUM]
                            elif t < 4:
                                lhsT = wtA[:, j, :, t - 1, 0:LAYERNUM]
                            elif t < 8:
                                lhsT = wtB1[:, j, :, t - 4, 0:LAYERNUM]
                            else:
                                lhsT = wtB2[:, j, :, t - 8, 0:LAYERNUM]
                            if b == 0:
                                rhs = emb0j[j][:, i:SL, :].rearrange(
                                    "p s e -> p e s")
                            else:
                                rhs = embs[b][:, j, i:SL, :].rearrange(
                                    "p s e -> p e s")
                            nc.tensor.matmul(
                                ps[:, 0:SL - i],
                                lhsT=lhsT,
                                rhs=rhs,
                                start=(i == 0 and j == 0),
                                stop=(i == w - 1 and j == KC8 - 1),
                                perf_mode=mybir.MatmulPerfMode.DoubleRow,
                            )
                    nc.vector.reduce_max(pooled[wi][:, b:b + 1], ps[:],
                                         axis=mybir.AxisListType.X)
                    t0 += w

            # Queue-warmer: a tiny DMA gated on pooled[0]'s last write (~4us
            # before the end) keeps the sync DMA queue hot so the final out
            # transfer doesn't pay the cold doorbell->transfer latency.
            nc.sync.dma_start(scratch_d, pooled[0][:, 6:7], single_packet=True)

            fin = psump.tile([NB, DOUT], F32, tag="fin")
            for wi in range(3):
                pr = outp.tile([LAYERNUM, NB], BF16, tag=f"pr{wi}", name=f"pr{wi}")
                # relu((x + C*bias)) with C descaled via OL/C on host: one DVE op
                nc.vector.tensor_scalar(pr[:], pooled[wi][:],
                                        scalar1=bias_t[:, wi:wi + 1], scalar2=0.0,
                                        op0=mybir.AluOpType.add,
                                        op1=mybir.AluOpType.max)
                nc.tensor.matmul(fin[:], lhsT=pr[:], rhs=ol_t[:, wi, :],
                                 start=(wi == 0), stop=(wi == 2))
            res = outp.tile([NB, DOUT], F32)
            nc.vector.tensor_copy(res[:], fin[:])
            nc.sync.dma_start(out_d, res[:], single_packet=True)

    nc.compile()
    return nc


def kernel(words, Embedding, outputlayer, filters_w3, bias_w3,
           filters_w4, bias_w4, filters_w5, bias_w5):
    global LAST_RESULTS
    words = np.asarray(words)
    Embedding = np.asarray(Embedding, dtype=np.float32)
    outputlayer = np.asarray(outputlayer, dtype=np.float32)
    filts = {3: np.asarray(filters_w3, dtype=np.float32),
             4: np.asarray(filters_w4, dtype=np.float32),
             5: np.asarray(filters_w5, dtype=np.float32)}
    biases = {3: np.asarray(bias_w3, dtype=np.float32),
              4: np.asarray(bias_w4, dtype=np.float32),
              5: np.asarray(bias_w5, dtype=np.float32)}

    # Dedup referenced vocab, cast only the used rows to scaled fp8, then
    # host-gather every batch element into the DoubleRow pair layout.
    uniq, inv = np.unique(words, return_inverse=True)
    table = (Embedding[uniq] * np.float32(S_E)).astype(NPF8)
    inv = inv.reshape(B, SL)

    K_all = np.stack([filts[w].reshape(LAYERNUM, w, D)[:, i, :].T
                      for w in WIDTHS for i in range(w)])    # [12, 512, 100]
    K8 = np.clip(K_all * np.float32(S_K), -240, 240).astype(NPF8)
    # lhsT pair layout: [p, j, e, t, m] padded m->LPAD; t split 1+3+8
    wts_full = np.zeros((P, KC8, 2, NT, LPAD), dtype=NPF8)
    wts_full[:, :, :, :, :LAYERNUM] = \
        K8.reshape(NT, KC8, P, 2, LAYERNUM).transpose(2, 1, 3, 0, 4)
    wts0 = wts_full[:, :, :, :1].reshape(P, KC8 * 2 * 1 * LPAD).copy()
    wtsA = wts_full[:, :, :, 1:4].reshape(P, KC8 * 2 * 3 * LPAD).copy()
    wtsB1 = wts_full[:, :, :, 4:8].reshape(P, KC8 * 2 * 4 * LPAD).copy()
    wtsB2 = wts_full[:, :, :, 8:].reshape(P, KC8 * 2 * 4 * LPAD).copy()
    C = np.float32(S_E * S_K)
    ol = (outputlayer.reshape(3, LAYERNUM, DOUT).transpose(1, 0, 2) / C) \
        .astype(NPBF16).reshape(LAYERNUM, 3 * DOUT).copy()
    bias = (np.stack([biases[w] for w in WIDTHS], axis=1) * C).copy()

    in_maps = []
    for core in range(NCORES):
        ridx = inv[core * NB:(core + 1) * NB]
        g = table[ridx]                                       # [NB, SL, D]
        e = (g.reshape(NB, SL, KC8, P, 2).transpose(3, 0, 2, 1, 4)
             .reshape(P, NB * KC8 * SL * 2).copy())
        in_maps.append({"emb": e, "wts0": wts0, "wtsA": wtsA,
                        "wtsB1": wtsB1, "wtsB2": wtsB2,
                        "ol": ol, "bias": bias})

    nc = _CACHE.get("nc")
    if nc is None:
        nc = _CACHE["nc"] = _build()

    res = run_bass_kernel_spmd(nc, in_maps, core_ids=list(range(NCORES)))
    LAST_RESULTS = res
    return np.concatenate([res.results[i]["out"] for i in range(NCORES)],
                          axis=0).astype(np.float32)


# revision 39
# speedup vs baseline: 1.1702x; 1.1702x over previous
"""Trainium2 Bass kernel for CNNText: embedding gather + multi-width conv1d
+ bias/ReLU/max-pool + output matmul, data-parallel over batch on 8 NeuronCores.

Per core (8 batch elements):
  - Host: dedup words -> compact fp8(e4m3, x2^19) rows; ALL 8 batch elems'
    embeddings are host-pregathered into the DoubleRow pair layout (d-pairs
    per partition, K=256 per chunk), so the device needs no gather library,
    no Q7 ucode boot, and no gpsimd work at all.  Filters pre-transposed/
    scaled (x2^10) to fp8; scales fold back out in the ReLU's bias operand
    and the bf16 output layer (max-pool commutes with positive scaling).
  - Device: conv = PSUM-accumulated shifted matmuls (fp8 DoubleRow, ~216ns
    per K=512 x M=100 x N=512 pair, ~98% PE roofline); free-dim max reduce;
    relu(max+C*bias) -> bf16; [8,300]@[300,10] accumulated per width so the
    tail chain after the last conv matmul is one reduce+relu+matmul deep.
  - Startup: emb_b0 and the w3 filter tiles lead the two HWDGE queues so
    the first conv matmul can issue ~3us in; a short PE warmup burst keeps
    the HAM clock gate ramping while the first DMAs land.
"""
import numpy as np
import ml_dtypes
from contextlib import ExitStack

import concourse.tile as tile
from concourse import bacc, mybir
from concourse.bass_utils import run_bass_kernel_spmd

# This image's antenv lacks axon_hooks; if tracing is requested via
# BASS_TRACE, bass_utils imports it. Provide a null shim so the run
# degrades to no-trace instead of crashing.
try:
    import antenv.axon_hooks  # noqa: F401
except ImportError:
    import sys as _sys
    import types as _types
    _m = _types.ModuleType("antenv.axon_hooks")
    _m.get_axon_ntff_profile_hook = lambda: None
    _m.set_axon_ntff_profile_hook = lambda h: None
    _sys.modules["antenv.axon_hooks"] = _m

P = 128
SL = 512
D = 512
B = 64
NCORES = 8
NB = B // NCORES
LAYERNUM = 100
WIDTHS = [3, 4, 5]
NT = sum(WIDTHS)          # 12 (width, offset) filter tiles
KC8 = 2                   # contraction chunks of 256 (d-pairs per partition)
NWARM = 45                # PE warmup matmuls while the first input DMAs land
LPAD = 112                # filter dim padded so DR pair-dim strides are %16==0
DOUT = 10
S_E, S_K = 2.0**19, 2.0**10   # fp8 pre-scales for embedding / filters

F8 = mybir.dt.float8e4
F32 = mybir.dt.float32
BF16 = mybir.dt.bfloat16
NPF8 = ml_dtypes.float8_e4m3
NPBF16 = ml_dtypes.bfloat16

_CACHE: dict = {}
LAST_RESULTS = None


def _build():
    nc = bacc.Bacc("TRN2", target_bir_lowering=False, debug=False,
                   enable_asserts=True, num_devices=NCORES)

    emb_d = nc.dram_tensor("emb", [P, NB * KC8 * SL * 2], F8, kind="ExternalInput").ap()
    wts0_d = nc.dram_tensor("wts0", [P, KC8 * 2 * 1 * LPAD], F8, kind="ExternalInput").ap()
    wtsA_d = nc.dram_tensor("wtsA", [P, KC8 * 2 * 3 * LPAD], F8, kind="ExternalInput").ap()
    wtsB1_d = nc.dram_tensor("wtsB1", [P, KC8 * 2 * 4 * LPAD], F8, kind="ExternalInput").ap()
    wtsB2_d = nc.dram_tensor("wtsB2", [P, KC8 * 2 * 4 * LPAD], F8, kind="ExternalInput").ap()
    ol_d = nc.dram_tensor("ol", [LAYERNUM, 3 * DOUT], BF16, kind="ExternalInput").ap()
    bias_d = nc.dram_tensor("bias", [LAYERNUM, 3], F32, kind="ExternalInput").ap()
    out_d = nc.dram_tensor("out", [NB, DOUT], F32, kind="ExternalOutput").ap()
    scratch_d = nc.dram_tensor("scratch", [LAYERNUM, 1], F32, kind="ExternalOutput").ap()

    with tile.TileContext(nc) as tc:
        with ExitStack() as ctx:
            consts = ctx.enter_context(tc.tile_pool(name="consts", bufs=1))
            embp = ctx.enter_context(tc.tile_pool(name="emb", bufs=NB))
            psump = ctx.enter_context(tc.tile_pool(name="psum", bufs=2, space="PSUM"))
            outp = ctx.enter_context(tc.tile_pool(name="outp", bufs=1))

            emb_v = emb_d.rearrange("p (b j x) -> p b j x", b=NB, j=KC8)
            # b0's chunks are separate tiles so the first matmul is gated by
            # a 131KB transfer, not b0's full 262KB
            emb0j = [embp.tile([P, SL, 2], F8, tag=f"emb0j{j}", name=f"emb_b0j{j}")
                     for j in range(KC8)]
            embs = [None] + [embp.tile([P, KC8, SL, 2], F8, tag="emb",
                                       name=f"emb_b{b}") for b in range(1, NB)]
            # DR weight AP needs the pair-dim byte stride %16==0 (s3_lw.md,
            # checkMatmultPerfMode): LPAD=112 keeps 1/3/4 * 112 all %16==0
            wt0 = consts.tile([P, KC8, 2, 1, LPAD], F8)
            wtA = consts.tile([P, KC8, 2, 3, LPAD], F8)
            wtB1 = consts.tile([P, KC8, 2, 4, LPAD], F8)
            wtB2 = consts.tile([P, KC8, 2, 4, LPAD], F8)

            # Both HWDGE queues are loaded in consumption order of the conv
            # stream: emb_b0/j0 + the t=0 filter tile gate the first matmul;
            # the rest of the weight set is split across the queues so it
            # lands before b0's w4/w5 matmuls need it.
            nc.sync.dma_start(
                emb0j[0][:].rearrange("p s e -> p (s e)"), emb_v[:, 0, 0])
            nc.scalar.dma_start(wt0[:], wts0_d.rearrange(
                "p (j e t f) -> p j e t f", j=KC8, e=2, t=1))
            nc.sync.dma_start(
                emb0j[1][:].rearrange("p s e -> p (s e)"), emb_v[:, 0, 1])
            nc.scalar.dma_start(wtB1[:], wtsB1_d.rearrange(
                "p (j e t f) -> p j e t f", j=KC8, e=2, t=4))
            nc.sync.dma_start(wtA[:], wtsA_d.rearrange(
                "p (j e t f) -> p j e t f", j=KC8, e=2, t=3))
            nc.sync.dma_start(wtB2[:], wtsB2_d.rearrange(
                "p (j e t f) -> p j e t f", j=KC8, e=2, t=4))
            for b in range(1, NB):
                eng = nc.sync if b % 2 == 1 else nc.scalar
                eng.dma_start(
                    embs[b][:].rearrange("p j s e -> p (j s e)"),
                    emb_v[:, b].rearrange("p j x -> p (j x)"))
            ol_t = consts.tile([LAYERNUM, 3, DOUT], BF16)
            nc.scalar.dma_start(ol_t[:], ol_d.rearrange("p (w o) -> p w o", w=3))
            bias_t = consts.tile([LAYERNUM, 3], F32)
            nc.scalar.dma_start(bias_t[:], bias_d)

            pooled = [outp.tile([LAYERNUM, NB], F32, tag=f"pool{wi}", name=f"pool{wi}")
                      for wi in range(3)]

            # PE warmup: throwaway matmuls during the input-DMA wait keep the
            # HAM clock gate ramping toward 8/8 before the real stream starts.
            warm = consts.tile([P, P], F8, name="warm")
            nc.vector.memset(warm[:], 0)
            warm_ps = psump.tile([P, P], F32, tag="fin")
            for _ in range(NWARM):
                nc.tensor.matmul(warm_ps[:], lhsT=warm[:], rhs=warm[:],
                                 start=True, stop=True)

            for b in range(NB):
                t0 = 0
                for wi, w in enumerate(WIDTHS):
                    ps = psump.tile([LAYERNUM, SL], F32, tag=f"ps{wi}")
                    for i in range(w):
                        t = t0 + i
                        for j in range(KC8):
                            if t == 0:
                                lhsT = wt0[:, j, :, 0, 0:LAYERNUM]
                            elif t < 4:
                                lhsT = wtA[:, j, :, t - 1, 0:LAYERNUM]
                            elif t < 8:
                                lhsT = wtB1[:, j, :, t - 4, 0:LAYERNUM]
                            else:
                                lhsT = wtB2[:, j, :, t - 8, 0:LAYERNUM]
                            if b == 0:
                                rhs = emb0j[j][:, i:SL, :].rearrange(
                                    "p s e -> p e s")
                            else:
                                rhs = embs[b][:, j, i:SL, :].rearrange(
                                    "p s e -> p e s")
                            nc.tensor.matmul(
                                ps[:, 0:SL - i],
                                lhsT=lhsT,
                                rhs=rhs,
                                start=(i == 0 and j == 0),
                                stop=(i == w - 1 and j == KC8 - 1),
                                perf_mode=mybir.MatmulPerfMode.DoubleRow,
                            )
                    nc.vector.reduce_max(pooled[wi][:, b:b + 1], ps[:],
                                         axis=mybir.AxisListType.X)
                    t0 += w

            # Queue-warmer: a tiny DMA gated on pooled[0]'s last write (~4us
            # before the end) keeps the sync DMA queue hot so the final out
            # transfer doesn't pay the cold doorbell->transfer latency.
            nc.sync.dma_start(scratch_d, pooled[0][:, 6:7], single_packet=True)

            fin = psump.tile([NB, DOUT], F32, tag="fin")
            for wi in range(3):
                pr = outp.tile([LAYERNUM, NB], BF16, tag=f"pr{wi}", name=f"pr{wi}")
                # relu((x + C*bias)) with C descaled via OL/C on host: one DVE op
                nc.vector.tensor_scalar(pr[:], pooled[wi][:],
                                        scalar1=bias_t[:, wi:wi + 1], scalar2=0.0,
                                        op0=mybir.AluOpType.add,
                                        op1=mybir.AluOpType.max)
                nc.tensor.matmul(fin[:], lhsT=pr[:], rhs=ol_t[:, wi, :],
                                 start=(wi == 0), stop=(wi == 2))
            res = outp.tile([NB, DOUT], F32)
            nc.vector.tensor_copy(res[:], fin[:])
            nc.sync.dma_start(out_d, res[:], single_packet=True)

    nc.compile()
    return nc


def kernel(words, Embedding, outputlayer, filters_w3, bias_w3,
           filters_w4, bias_w4, filters_w5, bias_w5):
    global LAST_RESULTS
    words = np.asarray(words)
    Embedding = np.asarray(Embedding, dtype=np.float32)
    outputlayer = np.asarray(outputlayer, dtype=np.float32)
    filts = {3: np.asarray(filters_w3, dtype=np.float32),
             4: np.asarray(filters_w4, dtype=np.float32),
             5: np.asarray(filters_w5, dtype=np.float32)}
    biases = {3: np.asarray(bias_w3, dtype=np.float32),
              4: np.asarray(bias_w4, dtype=np.float32),
              5: np.asarray(bias_w5, dtype=np.float32)}

    # Dedup referenced vocab, cast only the used rows to scaled fp8, then
    # host-gather every batch element into the DoubleRow pair layout.
    uniq, inv = np.unique(words, return_inverse=True)
    table = (Embedding[uniq] * np.float32(S_E)).astype(NPF8)
    inv = inv.reshape(B, SL)

    K_all = np.stack([filts[w].reshape(LAYERNUM, w, D)[:, i, :].T
                      for w in WIDTHS for i in range(w)])    # [12, 512, 100]
    K8 = np.clip(K_all * np.float32(S_K), -240, 240).astype(NPF8)
    # lhsT pair layout: [p, j, e, t, m] padded m->LPAD; t split 1+3+8
    wts_full = np.zeros((P, KC8, 2, NT, LPAD), dtype=NPF8)
    wts_full[:, :, :, :, :LAYERNUM] = \
        K8.reshape(NT, KC8, P, 2, LAYERNUM).transpose(2, 1, 3, 0, 4)
    wts0 = wts_full[:, :, :, :1].reshape(P, KC8 * 2 * 1 * LPAD).copy()
    wtsA = wts_full[:, :, :, 1:4].reshape(P, KC8 * 2 * 3 * LPAD).copy()
    wtsB1 = wts_full[:, :, :, 4:8].reshape(P, KC8 * 2 * 4 * LPAD).copy()
    wtsB2 = wts_full[:, :, :, 8:].reshape(P, KC8 * 2 * 4 * LPAD).copy()
    C = np.float32(S_E * S_K)
    ol = (outputlayer.reshape(3, LAYERNUM, DOUT).transpose(1, 0, 2) / C) \
        .astype(NPBF16).reshape(LAYERNUM, 3 * DOUT).copy()
    bias = (np.stack([biases[w] for w in WIDTHS], axis=1) * C).copy()

    in_maps = []
    for core in range(NCORES):
        ridx = inv[core * NB:(core + 1) * NB]
        g = table[ridx]                                       # [NB, SL, D]
        e = (g.reshape(NB, SL, KC8, P, 2).transpose(3, 0, 2, 1, 4)
             .reshape(P, NB * KC8 * SL * 2).copy())
        in_maps.append({"emb": e, "wts0": wts0, "wtsA": wtsA,
                        "wtsB1": wtsB1, "wtsB2": wtsB2,
                        "ol": ol, "bias": bias})

    nc = _CACHE.get("nc")
    if nc is None:
        nc = _CACHE["nc"] = _build()

    res = run_bass_kernel_spmd(nc, in_maps, core_ids=list(range(NCORES)))
    LAST_RESULTS = res
    return np.concatenate([res.results[i]["out"] for i in range(NCORES)],
                          axis=0).astype(np.float32)
